# revision 2
# baseline (speedup 1.0000x reference)
"""Hybrid Trainium2 kernel for nn_DFFN (v2).

Fold blocks 0,1 (x1[0:128], x2[0:128]) on the PE (7 K-packed passes);
block2 (x1[128:192] | x2[128:192]) via s2 = W_in2 @ x (PE, K=96) + 3x3
depthwise conv on DVE/ScalarE/GPSIMD:
  - 5 tap-muls on DVE at 4x (aligned via a +1-shifted s2 copy made by GPSIMD)
  - 4 tap-muls on ScalarE (activation-Copy with per-partition scale)
  - 8 tree-adds per half-band on DVE
  - gate muls for block2 on GPSIMD
psA/psB are pair-consolidated (4-row, 2-bank PSUM tiles) so gelu/g0 evacs run
at FD=1024. s2 halo row 0 is reused from the previous band via SBUF DMA.
psD (proj_out) of band b is interleaved into band b+1's fold stream.
"""

import numpy as np
import ml_dtypes

B, CIN, H, W = 4, 96, 256, 256
C2, HID = 384, 192
N_CORES = 8
ROWS = (B * H) // N_CORES
RPT = 2
BAND = 16
WP = W + 8
BF16 = ml_dtypes.bfloat16

_compiled = {}


def _build_nc(rows):
    import concourse.bass as bass  # noqa: F401
    import concourse.tile as tile
    from concourse import bacc, mybir

    dt = mybir.dt
    AFT = mybir.ActivationFunctionType
    ALU = mybir.AluOpType

    nc = bacc.Bacc("TRN2", target_bir_lowering=False, debug=False,
                   num_devices=N_CORES)
    x_d = nc.dram_tensor("x", [CIN, rows + 2, W + 4], dt.bfloat16,
                         kind="ExternalInput").ap()
    w7_d = nc.dram_tensor("w7", [128, 7, 256], dt.bfloat16,
                          kind="ExternalInput").ap()
    wi2_d = nc.dram_tensor("wi2", [CIN, 128], dt.bfloat16,
                           kind="ExternalInput").ap()
    kdw_d = nc.dram_tensor("kdw", [128, 9], dt.float32,
                           kind="ExternalInput").ap()
    wo_d = nc.dram_tensor("wo", [HID, CIN], dt.bfloat16,
                          kind="ExternalInput").ap()
    y_d = nc.dram_tensor("y", [CIN, rows, W], dt.float32,
                         kind="ExternalOutput").ap()

    if rows == 128:
        bands = [4, 12] + [BAND] * 6 + [12, 4]
    elif rows >= 24:
        bands = [8] + [BAND] * ((rows - 16) // BAND) + [8]
    else:
        bands = [8, rows - 8] if rows > 8 else [rows]
    assert sum(bands) == rows

    # taps: t = 3*dy + dx. DVE taps: dx==1 from s2b (offset 2, aligned) and
    # (0,0),(2,0) from the +1-shifted copy (offset 0). ScalarE: the rest.
    DVE_T = (0, 2, 3, 5, 6, 8)  # dx in {0,2}: s2s col offset 2+dx, aligned
    SE_T = (1, 4, 7)            # dx == 1: s2s col offset 3 (ScalarE, 1x)

    with tile.TileContext(nc) as tc:
        with (
            tc.tile_pool(name="consts", bufs=1) as consts,
            tc.tile_pool(name="xk", bufs=2) as xkp,
            tc.tile_pool(name="s2p", bufs=2) as s2p,
            tc.tile_pool(name="s2s", bufs=2) as s2sp,
            tc.tile_pool(name="tt", bufs=5) as ttp,
            tc.tile_pool(name="ch", bufs=2) as chp,
            tc.tile_pool(name="gap", bufs=1) as gap_p,
            tc.tile_pool(name="gate", bufs=2) as gatep,
            tc.tile_pool(name="tga", bufs=3) as tgap,
            tc.tile_pool(name="otp", bufs=3) as otp,
            tc.tile_pool(name="gb", bufs=2) as gbp,
            tc.tile_pool(name="psA", bufs=1, space="PSUM") as psA_pool,
            tc.tile_pool(name="psB", bufs=1, space="PSUM") as psB_pool,
            tc.tile_pool(name="psS", bufs=2, space="PSUM") as psS_pool,
            tc.tile_pool(name="psD", bufs=2, space="PSUM") as psD_pool,
        ):
            W7s = consts.tile([128, 7, 256], dt.bfloat16)
            nc.sync.dma_start(W7s[:], w7_d[:])
            wi2 = consts.tile([CIN, 128], dt.bfloat16)
            nc.sync.dma_start(wi2[:], wi2_d[:])
            kdw = consts.tile([128, 9], dt.float32)
            nc.sync.dma_start(kdw[:], kdw_d[:])
            woA = consts.tile([128, CIN], dt.bfloat16)
            nc.sync.dma_start(woA[:], wo_d[0:128, :])
            woB = consts.tile([64, CIN], dt.bfloat16)
            nc.sync.dma_start(woB[:], wo_d[128:HID, :])
            woBh = consts.tile([128, CIN], dt.bfloat16)
            nc.sync.dma_start(woBh[64:128], wo_d[128:HID, :])

            scratch = consts.tile([128, 512], dt.bfloat16)
            nc.vector.memset(scratch[:], 0.0)
            warm = psD_pool.tile([CIN, RPT, W], dt.float32, tag="psD")
            for _ in range(14):
                nc.tensor.matmul(warm[:], scratch[:, 0:CIN], scratch[:],
                                 start=True, stop=True)

            pending = []
            prev_s2s = [None, 0]

            def emit_psD_pair(g0b, g1b, r0_, t0):
                t1 = t0 + RPT
                psD = [psD_pool.tile([CIN, RPT, W], dt.float32, tag="psD",
                                     name=f"psD{r0_}_{t0}_{u}")
                       for u in range(2)]
                nc.tensor.matmul(psD[0][:], woB[:],
                                 g1b[0:64, t0:t0 + RPT, :],
                                 start=True, stop=False)
                nc.tensor.matmul(psD[1][:], woBh[64:128, :],
                                 g1b[64:128, t1:t1 + RPT, :],
                                 start=True, stop=False, tile_position=(64, 0))
                for u, tt_ in ((0, t0), (1, t1)):
                    nc.tensor.matmul(psD[u][:], woA[:],
                                     g0b[:, tt_:tt_ + RPT, :],
                                     start=False, stop=True)
                for u, tt_ in ((0, t0), (1, t1)):
                    ot = otp.tile([CIN, RPT, W], dt.float32, tag="ot",
                                   name=f"ot{r0_}_{tt_}")
                    nc.scalar.activation(ot[:], psD[u][:], AFT.Copy)
                    nc.sync.dma_start(
                        y_d[:, r0_ + tt_: r0_ + tt_ + RPT, :], ot[:])

            r0 = 0
            for bidx, band_rows in enumerate(bands):
                hb = band_rows + 2
                n_s2 = hb // 2
                halves = [(0, 8), (8, band_rows)] if band_rows > 8 else \
                         [(0, band_rows)]

                xk0 = xkp.tile([128, BAND + 2, W + 4], dt.bfloat16, tag="xk0")
                nc.sync.dma_start(xk0[0:96, 0:hb],
                                  x_d[0:96, r0: r0 + hb, :])
                nc.sync.dma_start(xk0[96:128, 0:band_rows],
                                  x_d[0:32, r0 + 1: r0 + 1 + band_rows, :])
                xk1 = xkp.tile([128, BAND, W + 4], dt.bfloat16, tag="xk1")
                nc.sync.dma_start(xk1[0:64, 0:band_rows],
                                  x_d[32:96, r0 + 1: r0 + 1 + band_rows, :])
                nc.sync.dma_start(xk1[64:128, 0:band_rows],
                                  x_d[0:64, r0 + 2: r0 + 2 + band_rows, :])
                xk2 = xkp.tile([96, BAND, W + 4], dt.bfloat16, tag="xk2")
                for i, dxb in enumerate((-1, 0, 1)):
                    nc.sync.dma_start(
                        xk2[32 * i: 32 * i + 32, 0:band_rows, 2: 2 + W],
                        x_d[64:96, r0 + 2: r0 + 2 + band_rows,
                            2 + dxb: 2 + dxb + W])

                s2s = s2sp.tile([128, BAND + 2, WP], dt.bfloat16, tag="s2s")
                if bidx < 2:
                    # ring buffers: borders stay zero; data writes never
                    # touch cols <3 or >=3+W
                    nc.gpsimd.memset(s2s[:, :, 0:3], 0.0)
                    nc.gpsimd.memset(s2s[:, :, 3 + W: WP], 0.0)

                def s2_gemm(st):
                    psS = psS_pool.tile([128, RPT, W], dt.float32, tag="psS",
                                        name=f"psS{r0}_{st}")
                    nc.tensor.matmul(psS[:], wi2[:],
                                     xk0[0:96, 2 * st: 2 * st + 2, 2: 2 + W],
                                     start=True, stop=True)
                    nc.scalar.activation(s2s[:, 2 * st: 2 * st + 2, 3: 3 + W],
                                         psS[:], AFT.Copy)

                passes = [(xk0, 1), (xk0, 2), (xk0, 3),
                          (xk1, 1), (xk1, 2), (xk1, 3), (xk2, 2)]
                g0b = gbp.tile([128, BAND, W], dt.bfloat16, tag="g0b")
                g1b = gbp.tile([128, BAND, W], dt.bfloat16, tag="g1b")

                def fold_pair(pj):
                    t0 = 2 * RPT * pj
                    if t0 >= band_rows:
                        return
                    nr = min(2 * RPT, band_rows - t0)
                    psA = psA_pool.tile([128, 2 * RPT, W], dt.float32,
                                        tag="psA", name=f"psA{r0}_{pj}")
                    psB = psB_pool.tile([128, 2 * RPT, W], dt.float32,
                                        tag="psB", name=f"psB{r0}_{pj}")
                    for m, ps in ((0, psA), (1, psB)):
                        for p, (xk, off) in enumerate(passes):
                            kk = xk.shape[0]
                            for u0 in range(0, nr, RPT):
                                nc.tensor.matmul(
                                    ps[:, u0: u0 + RPT, :],
                                    W7s[0:kk, p, 128 * m: 128 * (m + 1)],
                                    xk[:, t0 + u0: t0 + u0 + RPT,
                                       off: off + W],
                                    start=(p == 0), stop=(p == 6))
                    tga = tgap.tile([128, 2 * RPT, W], dt.bfloat16, tag="tga",
                                    name=f"tga{r0}_{pj}")
                    nc.scalar.activation(tga[:, 0:nr], psA[:, 0:nr], AFT.Gelu)
                    nc.vector.tensor_mul(g0b[:, t0: t0 + nr, :],
                                         tga[:, 0:nr], psB[:, 0:nr])

                def tap_mul(t, h0, h1, src_tile, off, rbase=None):
                    dy, dx = divmod(t, 3)
                    n = h1 - h0
                    rb = h0 if rbase is None else rbase
                    src = src_tile[:, dy + rb: dy + rb + n, off: off + W]
                    tt_ = ttp.tile([128, BAND, W], dt.bfloat16, tag="tt",
                                   name=f"tt{r0}_{t}_{h0}")
                    if t in SE_T:
                        nc.scalar.activation(tt_[:, 0:n], src, AFT.Copy,
                                             scale=kdw[:, t:t + 1])
                    else:
                        nc.vector.tensor_scalar_mul(tt_[:, 0:n], src,
                                                    kdw[:, t:t + 1])
                    return tt_

                def gate_half(h0, h1, acch):
                    n = h1 - h0
                    tgb = gatep.tile([64, 8, W], dt.bfloat16, tag="tgb",
                                     name=f"tgb{r0}_{h0}")
                    nc.scalar.activation(tgb[:, 0:n], acch[0:64, h0:h1],
                                         AFT.Gelu)
                    tx2l = gatep.tile([64, 8, W], dt.bfloat16, tag="tx2l",
                                      name=f"tx2l{r0}_{h0}")
                    nc.sync.dma_start(tx2l[:, 0:n], acch[64:128, h0:h1])
                    nc.vector.tensor_mul(g1b[0:64, h0:h1, :], tgb[:, 0:n],
                                         tx2l[:, 0:n])
                    tgbh = gatep.tile([128, 8, W], dt.bfloat16, tag="tgbh",
                                      name=f"tgbh{r0}_{h0}")
                    nc.sync.dma_start(tgbh[64:128, 0:n], tgb[:, 0:n])
                    nc.vector.tensor_mul(g1b[64:128, h0:h1, :],
                                         tgbh[64:128, 0:n], acch[64:128, h0:h1])

                prev = pending.pop() if pending else None
                prev_t0s = iter(range(0, prev[3], 2 * RPT)) if prev \
                    else iter(())

                def emit_next_psd():
                    if prev is None:
                        return
                    t0 = next(prev_t0s, None)
                    if t0 is not None:
                        emit_psD_pair(prev[0], prev[1], prev[2], t0)

                # ---- interleaved emission ----
                if prev_s2s[0] is not None:
                    pb = prev_s2s[1]
                    nc.sync.dma_start(s2s[:, 0:2, 3: 3 + W],
                                      prev_s2s[0][:, pb: pb + 2, 3: 3 + W])
                    s2_start = 1
                else:
                    s2_start = 0
                for st in range(s2_start, n_s2):
                    s2_gemm(st)

                n = band_rows
                T0 = {}
                for t in (0, 2, 3, 5, 6):
                    T0[t] = tap_mul(t, 0, band_rows, s2s, 2 + (t % 3))

                fold_pair(0)
                def chain_add(tag_i, a, b):
                    c = chp.tile([128, BAND, W], dt.bfloat16, tag="chain",
                                 name=f"ch{r0}_{tag_i}")
                    nc.vector.tensor_add(c[:, 0:n], a[:, 0:n], b[:, 0:n])
                    return c
                c0 = chain_add("0a", T0[0], T0[2])
                c0 = chain_add("0b", c0, T0[3])
                c0 = chain_add("0c", c0, T0[5])
                c0 = chain_add("0d", c0, T0[6])
                T0[8] = tap_mul(8, 0, band_rows, s2s, 2 + (8 % 3))
                c0 = chain_add("0e", c0, T0[8])
                T0[1] = tap_mul(1, 0, band_rows, s2s, 3)
                c0 = chain_add("0f", c0, T0[1])

                fold_pair(1)
                emit_next_psd()
                T0[4] = tap_mul(4, 0, band_rows, s2s, 3)
                c0 = chain_add("0g", c0, T0[4])
                T0[7] = tap_mul(7, 0, band_rows, s2s, 3)
                accb = chain_add("0h", c0, T0[7])
                gate_half(halves[0][0], halves[0][1], accb)

                fold_pair(2)
                emit_next_psd()

                if len(halves) > 1:
                    gate_half(halves[1][0], halves[1][1], accb)
                    fold_pair(3)
                    emit_next_psd()

                while True:
                    t0 = next(prev_t0s, None) if prev else None
                    if t0 is None:
                        break
                    emit_psD_pair(prev[0], prev[1], prev[2], t0)

                prev_s2s[0] = s2s
                prev_s2s[1] = band_rows
                pending.append((g0b, g1b, r0, band_rows))
                r0 += band_rows

            prev = pending.pop()
            for tt_ in range(0, prev[3], 2 * RPT):
                emit_psD_pair(prev[0], prev[1], prev[2], tt_)

    nc.compile()
    return nc


def _get_nc(rows=ROWS):
    if rows not in _compiled:
        _compiled[rows] = _build_nc(rows)
    return _compiled[rows]


def _host_prep(x, w_in, w_dw, w_out):
    perm01 = np.concatenate([np.arange(0, 128), np.arange(HID, HID + 128)])
    perm2 = np.concatenate([np.arange(128, HID), np.arange(HID + 128, C2)])

    w2 = w_in[:, :, None, None] * w_dw[:, 0][:, None]
    w2p = w2[perm01]

    w7 = np.zeros((128, 7, 256), np.float32)
    for wdw in range(2):
        for dxi in range(3):
            p = 3 * wdw + dxi
            for q in range(128):
                a = 128 * wdw + q
                if a >= 288:
                    break
                dyi, ch = divmod(a, CIN)
                w7[q, p, :] = w2p[:, ch, dyi, dxi]
    for q in range(96):
        dxi, c2i = divmod(q, 32)
        w7[q, 6, :] = w2p[:, 64 + c2i, 2, dxi]
    w7_sb = w7.astype(BF16)

    wi2 = np.ascontiguousarray(w_in[perm2].T).astype(BF16)
    kdw = np.ascontiguousarray(
        w_dw[perm2, 0].reshape(128, 9)).astype(np.float32)
    wo_sb = np.ascontiguousarray(w_out.T).astype(BF16)

    xpad = np.pad(x, ((0, 0), (0, 0), (1, 1), (2, 2))).astype(BF16)
    in_maps = []
    for k in range(N_CORES):
        b, r0 = k // 2, (k % 2) * ROWS
        in_maps.append({
            "x": np.ascontiguousarray(xpad[b, :, r0: r0 + ROWS + 2, :]),
            "w7": w7_sb,
            "wi2": wi2,
            "kdw": kdw,
            "wo": wo_sb,
        })
    return in_maps


def _run_device(x, w_in, w_dw, w_out, trace=False):
    from concourse.bass_utils import run_bass_kernel_spmd

    nc = _get_nc()
    in_maps = _host_prep(x, w_in, w_dw, w_out)
    res = run_bass_kernel_spmd(nc, in_maps, list(range(N_CORES)), trace=trace)
    out = np.empty((B, CIN, H, W), np.float32)
    for k in range(N_CORES):
        b, r0 = k // 2, (k % 2) * ROWS
        out[b, :, r0: r0 + ROWS, :] = res.results[k]["y"]
    return out, res


def _numpy_fallback(x, w_in, w_dw, fft_w, w_out):
    from scipy.special import erf

    P = 8
    y = np.einsum("oc,bchw->bohw", w_in, x, optimize=True)
    Bs, C, Hs, Ws = y.shape
    h, w = Hs // P, Ws // P
    yp = y.reshape(Bs, C, h, P, w, P).transpose(0, 1, 2, 4, 3, 5)
    yf = np.fft.rfft2(yp) * fft_w[None]
    yp = np.fft.irfft2(yf, s=(P, P))
    y = yp.transpose(0, 1, 2, 4, 3, 5).reshape(Bs, C, Hs, Ws)
    ypad = np.pad(y, ((0, 0), (0, 0), (1, 1), (1, 1)))
    y2 = np.zeros_like(y)
    for dy in range(3):
        for dx in range(3):
            y2 += w_dw[None, :, 0, dy, dx, None, None] * \
                ypad[:, :, dy: dy + Hs, dx: dx + Ws]
    x1, x2 = y2[:, :HID], y2[:, HID:]
    g = x1 * 0.5 * (1.0 + erf(x1 / np.sqrt(2.0))) * x2
    return np.einsum("oc,bchw->bohw", w_out, g, optimize=True).astype(np.float32)


def kernel(x, w_in, w_dw, fft_w, w_out):
    x = np.asarray(x, np.float32)
    w_in = np.asarray(w_in, np.float32)
    w_dw = np.asarray(w_dw, np.float32)
    fft_w = np.asarray(fft_w, np.float32)
    w_out = np.asarray(w_out, np.float32)
    if not np.all(fft_w == 1.0):
        return _numpy_fallback(x, w_in, w_dw, fft_w, w_out)
    out, _ = _run_device(x, w_in, w_dw, w_out)
    return out


# revision 3
# speedup vs baseline: 1.0895x; 1.0895x over previous
"""Hybrid Trainium2 kernel for nn_DFFN (v2).

Fold blocks 0,1 (x1[0:128], x2[0:128]) on the PE (7 K-packed passes);
block2 (x1[128:192] | x2[128:192]) via s2 = W_in2 @ x (PE, K=96) + 3x3
depthwise conv on DVE/ScalarE/GPSIMD:
  - 5 tap-muls on DVE at 4x (aligned via a +1-shifted s2 copy made by GPSIMD)
  - 4 tap-muls on ScalarE (activation-Copy with per-partition scale)
  - 8 tree-adds per half-band on DVE
  - gate muls for block2 on GPSIMD
psA/psB are pair-consolidated (4-row, 2-bank PSUM tiles) so gelu/g0 evacs run
at FD=1024. s2 halo row 0 is reused from the previous band via SBUF DMA.
psD (proj_out) of band b is interleaved into band b+1's fold stream.
"""

import numpy as np
import ml_dtypes

B, CIN, H, W = 4, 96, 256, 256
C2, HID = 384, 192
N_CORES = 8
ROWS = (B * H) // N_CORES
RPT = 2
BAND = 16
WP = W + 8
BF16 = ml_dtypes.bfloat16

_compiled = {}


def _build_nc(rows):
    import concourse.bass as bass  # noqa: F401
    import concourse.tile as tile
    from concourse import bacc, mybir

    dt = mybir.dt
    AFT = mybir.ActivationFunctionType
    ALU = mybir.AluOpType

    nc = bacc.Bacc("TRN2", target_bir_lowering=False, debug=False,
                   num_devices=N_CORES)
    x_d = nc.dram_tensor("x", [CIN, rows + 2, W + 4], dt.bfloat16,
                         kind="ExternalInput").ap()
    w7_d = nc.dram_tensor("w7", [128, 7, 256], dt.bfloat16,
                          kind="ExternalInput").ap()
    wi2_d = nc.dram_tensor("wi2", [CIN, 128], dt.bfloat16,
                           kind="ExternalInput").ap()
    kdw_d = nc.dram_tensor("kdw", [128, 9], dt.float32,
                           kind="ExternalInput").ap()
    wo_d = nc.dram_tensor("wo", [HID, CIN], dt.bfloat16,
                          kind="ExternalInput").ap()
    y_d = nc.dram_tensor("y", [CIN, rows, W], dt.float32,
                         kind="ExternalOutput").ap()

    if rows == 128:
        bands = [4, 12] + [BAND] * 6 + [12, 4]
    elif rows >= 24:
        bands = [8] + [BAND] * ((rows - 16) // BAND) + [8]
    else:
        bands = [8, rows - 8] if rows > 8 else [rows]
    assert sum(bands) == rows

    # taps: t = 3*dy + dx. DVE taps: dx==1 from s2b (offset 2, aligned) and
    # (0,0),(2,0) from the +1-shifted copy (offset 0). ScalarE: the rest.
    DVE_T = (0, 2, 3, 5, 6, 8)  # dx in {0,2}: s2s col offset 2+dx, aligned
    SE_T = (1, 4, 7)            # dx == 1: s2s col offset 3 (ScalarE, 1x)

    with tile.TileContext(nc) as tc:
        with (
            tc.tile_pool(name="consts", bufs=1) as consts,
            tc.tile_pool(name="xk", bufs=2) as xkp,
            tc.tile_pool(name="s2p", bufs=2) as s2p,
            tc.tile_pool(name="s2s", bufs=2) as s2sp,
            tc.tile_pool(name="tt", bufs=5) as ttp,
            tc.tile_pool(name="ch", bufs=2) as chp,
            tc.tile_pool(name="gap", bufs=1) as gap_p,
            tc.tile_pool(name="gate", bufs=2) as gatep,
            tc.tile_pool(name="tga", bufs=3) as tgap,
            tc.tile_pool(name="otp", bufs=3) as otp,
            tc.tile_pool(name="gb", bufs=2) as gbp,
            tc.tile_pool(name="psA", bufs=2, space="PSUM") as psA_pool,
            tc.tile_pool(name="psB", bufs=2, space="PSUM") as psB_pool,
            tc.tile_pool(name="psS", bufs=2, space="PSUM") as psS_pool,
            tc.tile_pool(name="psD", bufs=2, space="PSUM") as psD_pool,
        ):
            W7s = consts.tile([128, 7, 256], dt.bfloat16)
            nc.sync.dma_start(W7s[:], w7_d[:])
            wi2 = consts.tile([CIN, 128], dt.bfloat16)
            nc.sync.dma_start(wi2[:], wi2_d[:])
            kdw = consts.tile([128, 9], dt.float32)
            nc.sync.dma_start(kdw[:], kdw_d[:])
            woA = consts.tile([128, CIN], dt.bfloat16)
            nc.sync.dma_start(woA[:], wo_d[0:128, :])
            woB = consts.tile([64, CIN], dt.bfloat16)
            nc.sync.dma_start(woB[:], wo_d[128:HID, :])
            woBh = consts.tile([128, CIN], dt.bfloat16)
            nc.sync.dma_start(woBh[64:128], wo_d[128:HID, :])

            scratch = consts.tile([128, 512], dt.bfloat16)
            nc.vector.memset(scratch[:], 0.0)
            warm = psD_pool.tile([CIN, RPT, W], dt.float32, tag="psD")
            for _ in range(14):
                nc.tensor.matmul(warm[:], scratch[:, 0:CIN], scratch[:],
                                 start=True, stop=True)

            pending = []
            prev_s2s = [None, 0]

            def emit_psD_pair(g0b, g1b, r0_, t0):
                t1 = t0 + RPT
                psD = [psD_pool.tile([CIN, RPT, W], dt.float32, tag="psD",
                                     name=f"psD{r0_}_{t0}_{u}")
                       for u in range(2)]
                nc.tensor.matmul(psD[0][:], woB[:],
                                 g1b[0:64, t0:t0 + RPT, :],
                                 start=True, stop=False)
                nc.tensor.matmul(psD[1][:], woBh[64:128, :],
                                 g1b[64:128, t1:t1 + RPT, :],
                                 start=True, stop=False, tile_position=(64, 0))
                for u, tt_ in ((0, t0), (1, t1)):
                    nc.tensor.matmul(psD[u][:], woA[:],
                                     g0b[:, tt_:tt_ + RPT, :],
                                     start=False, stop=True)
                for u, tt_ in ((0, t0), (1, t1)):
                    ot = otp.tile([CIN, RPT, W], dt.float32, tag="ot",
                                   name=f"ot{r0_}_{tt_}")
                    nc.scalar.activation(ot[:], psD[u][:], AFT.Copy)
                    nc.sync.dma_start(
                        y_d[:, r0_ + tt_: r0_ + tt_ + RPT, :], ot[:])

            r0 = 0
            for bidx, band_rows in enumerate(bands):
                hb = band_rows + 2
                n_s2 = hb // 2
                halves = [(0, 8), (8, band_rows)] if band_rows > 8 else \
                         [(0, band_rows)]

                xk0 = xkp.tile([128, BAND + 2, W + 4], dt.bfloat16, tag="xk0")
                nc.sync.dma_start(xk0[0:96, 0:hb],
                                  x_d[0:96, r0: r0 + hb, :])
                nc.sync.dma_start(xk0[96:128, 0:band_rows],
                                  x_d[0:32, r0 + 1: r0 + 1 + band_rows, :])
                xk1 = xkp.tile([128, BAND, W + 4], dt.bfloat16, tag="xk1")
                nc.sync.dma_start(xk1[0:64, 0:band_rows],
                                  x_d[32:96, r0 + 1: r0 + 1 + band_rows, :])
                nc.sync.dma_start(xk1[64:128, 0:band_rows],
                                  x_d[0:64, r0 + 2: r0 + 2 + band_rows, :])
                xk2 = xkp.tile([96, BAND, W + 4], dt.bfloat16, tag="xk2")
                for i, dxb in enumerate((-1, 0, 1)):
                    nc.sync.dma_start(
                        xk2[32 * i: 32 * i + 32, 0:band_rows, 2: 2 + W],
                        x_d[64:96, r0 + 2: r0 + 2 + band_rows,
                            2 + dxb: 2 + dxb + W])

                s2s = s2sp.tile([128, BAND + 2, WP], dt.bfloat16, tag="s2s")
                if bidx < 2:
                    # ring buffers: borders stay zero; data writes never
                    # touch cols <3 or >=3+W
                    nc.gpsimd.memset(s2s[:, :, 0:3], 0.0)
                    nc.gpsimd.memset(s2s[:, :, 3 + W: WP], 0.0)

                def s2_gemm(st):
                    psS = psS_pool.tile([128, RPT, W], dt.float32, tag="psS",
                                        name=f"psS{r0}_{st}")
                    nc.tensor.matmul(psS[:], wi2[:],
                                     xk0[0:96, 2 * st: 2 * st + 2, 2: 2 + W],
                                     start=True, stop=True)
                    nc.scalar.activation(s2s[:, 2 * st: 2 * st + 2, 3: 3 + W],
                                         psS[:], AFT.Copy)

                passes = [(xk0, 1), (xk0, 2), (xk0, 3),
                          (xk1, 1), (xk1, 2), (xk1, 3), (xk2, 2)]
                g0b = gbp.tile([128, BAND, W], dt.bfloat16, tag="g0b")
                g1b = gbp.tile([128, BAND, W], dt.bfloat16, tag="g1b")

                def fold_pair(pj):
                    for tj in (2 * pj, 2 * pj + 1):
                        t0 = RPT * tj
                        if t0 >= band_rows:
                            return
                        psA = psA_pool.tile([128, RPT, W], dt.float32,
                                            tag="psA", name=f"psA{r0}_{tj}")
                        psB = psB_pool.tile([128, RPT, W], dt.float32,
                                            tag="psB", name=f"psB{r0}_{tj}")
                        for m, ps in ((0, psA), (1, psB)):
                            for p, (xk, off) in enumerate(passes):
                                kk = xk.shape[0]
                                nc.tensor.matmul(
                                    ps[:],
                                    W7s[0:kk, p, 128 * m: 128 * (m + 1)],
                                    xk[:, t0: t0 + RPT, off: off + W],
                                    start=(p == 0), stop=(p == 6))
                        tga = tgap.tile([128, RPT, W], dt.bfloat16, tag="tga",
                                        name=f"tga{r0}_{tj}")
                        nc.scalar.activation(tga[:], psA[:], AFT.Gelu)
                        nc.vector.tensor_mul(g0b[:, t0: t0 + RPT, :],
                                             tga[:], psB[:])

                def tap_mul(t, h0, h1, src_tile, off, rbase=None):
                    dy, dx = divmod(t, 3)
                    n = h1 - h0
                    rb = h0 if rbase is None else rbase
                    src = src_tile[:, dy + rb: dy + rb + n, off: off + W]
                    tt_ = ttp.tile([128, BAND, W], dt.bfloat16, tag="tt",
                                   name=f"tt{r0}_{t}_{h0}")
                    if t in SE_T:
                        nc.scalar.activation(tt_[:, 0:n], src, AFT.Copy,
                                             scale=kdw[:, t:t + 1])
                    else:
                        nc.vector.tensor_scalar_mul(tt_[:, 0:n], src,
                                                    kdw[:, t:t + 1])
                    return tt_

                def gate_half(h0, h1, acch):
                    n = h1 - h0
                    tgb = gatep.tile([64, 8, W], dt.bfloat16, tag="tgb",
                                     name=f"tgb{r0}_{h0}")
                    nc.scalar.activation(tgb[:, 0:n], acch[0:64, h0:h1],
                                         AFT.Gelu)
                    tx2l = gatep.tile([64, 8, W], dt.bfloat16, tag="tx2l",
                                      name=f"tx2l{r0}_{h0}")
                    nc.sync.dma_start(tx2l[:, 0:n], acch[64:128, h0:h1])
                    nc.vector.tensor_mul(g1b[0:64, h0:h1, :], tgb[:, 0:n],
                                         tx2l[:, 0:n])
                    tgbh = gatep.tile([128, 8, W], dt.bfloat16, tag="tgbh",
                                      name=f"tgbh{r0}_{h0}")
                    nc.sync.dma_start(tgbh[64:128, 0:n], tgb[:, 0:n])
                    nc.vector.tensor_mul(g1b[64:128, h0:h1, :],
                                         tgbh[64:128, 0:n], acch[64:128, h0:h1])

                prev = pending.pop() if pending else None
                prev_t0s = iter(range(0, prev[3], 2 * RPT)) if prev \
                    else iter(())

                def emit_next_psd():
                    if prev is None:
                        return
                    t0 = next(prev_t0s, None)
                    if t0 is not None:
                        emit_psD_pair(prev[0], prev[1], prev[2], t0)

                # ---- interleaved emission ----
                if prev_s2s[0] is not None:
                    pb = prev_s2s[1]
                    nc.sync.dma_start(s2s[:, 0:2, 3: 3 + W],
                                      prev_s2s[0][:, pb: pb + 2, 3: 3 + W])
                    s2_start = 1
                else:
                    s2_start = 0
                for st in range(s2_start, n_s2):
                    s2_gemm(st)

                n = band_rows
                T0 = {}
                for t in (0, 2, 3, 5, 6):
                    T0[t] = tap_mul(t, 0, band_rows, s2s, 2 + (t % 3))

                fold_pair(0)
                def chain_add(tag_i, a, b):
                    c = chp.tile([128, BAND, W], dt.bfloat16, tag="chain",
                                 name=f"ch{r0}_{tag_i}")
                    nc.vector.tensor_add(c[:, 0:n], a[:, 0:n], b[:, 0:n])
                    return c
                c0 = chain_add("0a", T0[0], T0[2])
                c0 = chain_add("0b", c0, T0[3])
                c0 = chain_add("0c", c0, T0[5])
                c0 = chain_add("0d", c0, T0[6])
                T0[8] = tap_mul(8, 0, band_rows, s2s, 2 + (8 % 3))
                c0 = chain_add("0e", c0, T0[8])
                T0[1] = tap_mul(1, 0, band_rows, s2s, 3)
                c0 = chain_add("0f", c0, T0[1])

                fold_pair(1)
                emit_next_psd()
                T0[4] = tap_mul(4, 0, band_rows, s2s, 3)
                c0 = chain_add("0g", c0, T0[4])
                T0[7] = tap_mul(7, 0, band_rows, s2s, 3)
                accb = chain_add("0h", c0, T0[7])
                gate_half(halves[0][0], halves[0][1], accb)

                fold_pair(2)
                emit_next_psd()

                if len(halves) > 1:
                    gate_half(halves[1][0], halves[1][1], accb)
                    fold_pair(3)
                    emit_next_psd()

                while True:
                    t0 = next(prev_t0s, None) if prev else None
                    if t0 is None:
                        break
                    emit_psD_pair(prev[0], prev[1], prev[2], t0)

                prev_s2s[0] = s2s
                prev_s2s[1] = band_rows
                pending.append((g0b, g1b, r0, band_rows))
                r0 += band_rows

            prev = pending.pop()
            for tt_ in range(0, prev[3], 2 * RPT):
                emit_psD_pair(prev[0], prev[1], prev[2], tt_)

    nc.compile()
    return nc


def _get_nc(rows=ROWS):
    if rows not in _compiled:
        _compiled[rows] = _build_nc(rows)
    return _compiled[rows]


def _host_prep(x, w_in, w_dw, w_out):
    perm01 = np.concatenate([np.arange(0, 128), np.arange(HID, HID + 128)])
    perm2 = np.concatenate([np.arange(128, HID), np.arange(HID + 128, C2)])

    w2 = w_in[:, :, None, None] * w_dw[:, 0][:, None]
    w2p = w2[perm01]

    w7 = np.zeros((128, 7, 256), np.float32)
    for wdw in range(2):
        for dxi in range(3):
            p = 3 * wdw + dxi
            for q in range(128):
                a = 128 * wdw + q
                if a >= 288:
                    break
                dyi, ch = divmod(a, CIN)
                w7[q, p, :] = w2p[:, ch, dyi, dxi]
    for q in range(96):
        dxi, c2i = divmod(q, 32)
        w7[q, 6, :] = w2p[:, 64 + c2i, 2, dxi]
    w7_sb = w7.astype(BF16)

    wi2 = np.ascontiguousarray(w_in[perm2].T).astype(BF16)
    kdw = np.ascontiguousarray(
        w_dw[perm2, 0].reshape(128, 9)).astype(np.float32)
    wo_sb = np.ascontiguousarray(w_out.T).astype(BF16)

    xpad = np.pad(x, ((0, 0), (0, 0), (1, 1), (2, 2))).astype(BF16)
    in_maps = []
    for k in range(N_CORES):
        b, r0 = k // 2, (k % 2) * ROWS
        in_maps.append({
            "x": np.ascontiguousarray(xpad[b, :, r0: r0 + ROWS + 2, :]),
            "w7": w7_sb,
            "wi2": wi2,
            "kdw": kdw,
            "wo": wo_sb,
        })
    return in_maps


def _run_device(x, w_in, w_dw, w_out, trace=False):
    from concourse.bass_utils import run_bass_kernel_spmd

    nc = _get_nc()
    in_maps = _host_prep(x, w_in, w_dw, w_out)
    res = run_bass_kernel_spmd(nc, in_maps, list(range(N_CORES)), trace=trace)
    out = np.empty((B, CIN, H, W), np.float32)
    for k in range(N_CORES):
        b, r0 = k // 2, (k % 2) * ROWS
        out[b, :, r0: r0 + ROWS, :] = res.results[k]["y"]
    return out, res


def _numpy_fallback(x, w_in, w_dw, fft_w, w_out):
    from scipy.special import erf

    P = 8
    y = np.einsum("oc,bchw->bohw", w_in, x, optimize=True)
    Bs, C, Hs, Ws = y.shape
    h, w = Hs // P, Ws // P
    yp = y.reshape(Bs, C, h, P, w, P).transpose(0, 1, 2, 4, 3, 5)
    yf = np.fft.rfft2(yp) * fft_w[None]
    yp = np.fft.irfft2(yf, s=(P, P))
    y = yp.transpose(0, 1, 2, 4, 3, 5).reshape(Bs, C, Hs, Ws)
    ypad = np.pad(y, ((0, 0), (0, 0), (1, 1), (1, 1)))
    y2 = np.zeros_like(y)
    for dy in range(3):
        for dx in range(3):
            y2 += w_dw[None, :, 0, dy, dx, None, None] * \
                ypad[:, :, dy: dy + Hs, dx: dx + Ws]
    x1, x2 = y2[:, :HID], y2[:, HID:]
    g = x1 * 0.5 * (1.0 + erf(x1 / np.sqrt(2.0))) * x2
    return np.einsum("oc,bchw->bohw", w_out, g, optimize=True).astype(np.float32)


def kernel(x, w_in, w_dw, fft_w, w_out):
    x = np.asarray(x, np.float32)
    w_in = np.asarray(w_in, np.float32)
    w_dw = np.asarray(w_dw, np.float32)
    fft_w = np.asarray(fft_w, np.float32)
    w_out = np.asarray(w_out, np.float32)
    if not np.all(fft_w == 1.0):
        return _numpy_fallback(x, w_in, w_dw, fft_w, w_out)
    out, _ = _run_device(x, w_in, w_dw, w_out)
    return out
